# revision 15
# baseline (speedup 1.0000x reference)
"""CrossDomainGAT Trainium2 kernel — gatherless edge-slot design.

Strategy (graph/data parallel per the sharding hint):
  - Destination nodes sharded across 8 cores (6250 dests/core, padded to
    6272 = 49 blocks x 128).  Edges are routed to the core owning the
    destination, so the per-edge softmax (over heads -- edge-local) and the
    scatter-add stay local.
  - NO on-device gather.  The host pre-gathers x^T into *edge-slot* order
    (dest-major rounds: slot s = r*128 + p holds the r-th in-edge of dest p
    of its block; dests are degree-sorted so blocks have uniform round
    counts).  The device computes Q and V *per edge slot* on the Tensor
    engine: per round, the 128-column x^T tile is the stationary operand and
    Wq^T / Wv^T stream through.  This replaces the baseline's
    dma_gather-based pipeline whose SWDGE descriptor generation saturated
    GpSimd (~8 ns/edge) and whose 512B-packet storm stalled DVE.
  - K for a block is computed once from the dest rows and broadcast across
    rounds via a stride-0 access pattern (dests are partition-aligned).
  - Per-edge math runs dest-major at BLOCK granularity (few, large ops):
    ACT drains Q PSUM; the Q*K product is split between DVE and the
    otherwise-idle Pool engine; the head-sum is a single 16->1 tensor_reduce;
    V's PSUM drain is fused into the probs multiply on DVE.
  - The scatter-add accumulates acc^T directly on the TensorEngine
    (stationary wv_r, streaming identity), so the output stage needs no
    transpose round-trip.  LN sqrt is deferred and batched; host un-permutes.
  - Emission is software-pipelined one block ahead (stage1: DMA + K + Q
    matmuls + drains; stage2: edge math + V + accumulation + output).
"""

import math
import numpy as np
import ml_dtypes

# ---------------------------------------------------------------- problem cfg
D = 128
H = 8
HD = 16
ALPHA = 0.2
LN_EPS = 1e-5

FULL_CFG = dict(
    N=50000,
    E=800000,
    NC=8,            # cores
    GROUP=4,         # rounds per PSUM bank group
    POOL_FRAC=0.45,  # fraction of prod rounds computed on the Pool engine
)


def host_prep(x, edge_index, edge_attr, cfg):
    """Route edges to dest cores, degree-sort dests into uniform blocks,
    and materialize x^T in edge-slot order (plus per-block dest tensors)."""
    N, E, NC = cfg["N"], cfg["E"], cfg["NC"]
    DPC = N // NC                      # dests per core
    NB = (DPC + 127) // 128            # dest blocks per core
    DPAD = NB * 128

    row = np.asarray(edge_index[0], dtype=np.int64)
    col = np.asarray(edge_index[1], dtype=np.int64)
    ea = np.asarray(edge_attr, dtype=np.float32)
    x = np.asarray(x, dtype=np.float32)
    bf = ml_dtypes.bfloat16

    core = col // DPC
    cl = col - core * DPC              # local dest id

    # ---- pass 1: per-core degree sort -> uniform per-block round counts
    perms, orders = [], []
    Rc = np.zeros((NC, NB), dtype=np.int64)
    for c in range(NC):
        clc = cl[core == c]
        deg = np.bincount(clc, minlength=DPC)
        order = np.argsort(-deg, kind="stable")
        dpad = np.concatenate([deg[order], np.zeros(DPAD - DPC, np.int64)])
        Rc[c] = dpad.reshape(NB, 128).max(axis=1)
        perm = np.concatenate([order, np.full(DPAD - DPC, -1, np.int64)])
        perms.append(perm)
        orders.append(order)

    R = np.maximum(Rc.max(axis=0), 1)  # uniform across cores, >=1
    r_off = np.concatenate([[0], np.cumsum(R)])
    NR = int(R.sum())                  # total rounds per core
    S = NR * 128                       # edge slots per core

    # x^T padded with one zero column for pad slots
    xT = np.ascontiguousarray(x.T).astype(bf)
    xTpad = np.concatenate([xT, np.zeros((D, 1), dtype=bf)], axis=1)

    xdt_arrs, ea_arrs, xtd_arrs, xd_arrs = [], [], [], []
    for c in range(NC):
        m = core == c
        clc = cl[m]
        rowc = row[m]
        eac = ea[m]
        order = orders[c]
        inv = np.empty(DPC, dtype=np.int64)
        inv[order] = np.arange(DPC)
        q = inv[clc]                   # dest slot of each edge
        b = q // 128
        p = q % 128
        # rank within dest via stable sort on dest slot
        sort = np.argsort(q, kind="stable")
        qs = q[sort]
        starts = np.r_[0, np.flatnonzero(np.diff(qs)) + 1]
        counts = np.diff(np.r_[starts, len(qs)])
        rank_sorted = np.arange(len(qs)) - np.repeat(starts, counts)
        rank = np.empty(len(qs), np.int64)
        rank[sort] = rank_sorted

        s_idx = (r_off[b] + rank) * 128 + p
        src_col = np.full(S, N, dtype=np.int64)  # default: the zero column
        src_col[s_idx] = rowc
        xdt_arrs.append(np.ascontiguousarray(xTpad[:, src_col]))

        ea_l = np.zeros((S, 16), dtype=np.float32)
        ea_l[s_idx] = eac
        ea_arrs.append(np.ascontiguousarray(
            ea_l.reshape(NR, 128, 16).transpose(1, 0, 2).reshape(128, NR * 16)
        ).astype(bf))

        perm = perms[c]
        xd = np.zeros((DPAD, D), dtype=np.float32)
        valid = perm >= 0
        xd[valid] = x[c * DPC + perm[valid]]
        xtd_arrs.append(np.ascontiguousarray(xd.T).astype(bf))
        xd_arrs.append(xd)  # bias bo is folded in by kernel() before upload

    meta = dict(
        cfg=cfg, DPC=DPC, NB=NB, DPAD=DPAD,
        R=R.astype(int).tolist(), NR=NR, S=S,
        r_off=r_off.astype(int).tolist(),
        n_rounds=NR,
    )
    arrs = dict(
        xdt=xdt_arrs, ea=ea_arrs, xtd=xtd_arrs, xd=xd_arrs, perms=perms,
    )
    return meta, arrs


# ------------------------------------------------------------------ weights
def host_weights(Wq, Wk, Wv, Wo, bo, gamma, beta):
    bf = ml_dtypes.bfloat16
    t = lambda W: np.ascontiguousarray(np.asarray(W, np.float32).T).astype(bf)
    rep = lambda v: np.tile(np.asarray(v, np.float32)[None, :], (128, 1))
    # 1/sqrt(HD) folded into Wk so K needs no on-device scale
    wk_s = np.asarray(Wk, np.float32) / math.sqrt(HD)
    return dict(
        wq_t=t(Wq), wk_t=t(wk_s), wv_t=t(Wv), wo_t=t(Wo),
        bo_b=rep(bo), gamma_b=rep(gamma), beta_b=rep(beta),
        ident=np.eye(128, dtype=np.float32).astype(bf),
    )


# ------------------------------------------------------------------ kernel IR
def build_nc(meta, debug=False):
    from contextlib import ExitStack
    import concourse.bacc as bacc
    import concourse.bass as bass
    import concourse.tile as tile
    from concourse import mybir

    cfg = meta["cfg"]
    NB, DPAD = meta["NB"], meta["DPAD"]
    R = meta["R"]
    NR, S = meta["NR"], meta["S"]
    r_off = meta["r_off"]
    GROUP = cfg["GROUP"]
    POOL_FRAC = cfg["POOL_FRAC"]
    RMAX = max(R)

    dt = mybir.dt
    AF = mybir.ActivationFunctionType
    AL = mybir.AluOpType

    nc = bacc.Bacc("TRN2", target_bir_lowering=False, debug=debug)

    # ---------- I/O ----------
    xdt_d = nc.dram_tensor("xdt", [128, S], dt.bfloat16, kind="ExternalInput")
    ea_d = nc.dram_tensor("ea", [128, NR * 16], dt.bfloat16, kind="ExternalInput")
    xtd_d = nc.dram_tensor("xtd", [128, DPAD], dt.bfloat16, kind="ExternalInput")
    xd_d = nc.dram_tensor("xd", [DPAD, 128], dt.float32, kind="ExternalInput")
    wq_d = nc.dram_tensor("wq_t", [128, 128], dt.bfloat16, kind="ExternalInput")
    wk_d = nc.dram_tensor("wk_t", [128, 128], dt.bfloat16, kind="ExternalInput")
    wv_d = nc.dram_tensor("wv_t", [128, 128], dt.bfloat16, kind="ExternalInput")
    wo_d = nc.dram_tensor("wo_t", [128, 128], dt.bfloat16, kind="ExternalInput")
    bo_d = nc.dram_tensor("bo_b", [128, 128], dt.float32, kind="ExternalInput")
    ga_d = nc.dram_tensor("gamma_b", [128, 128], dt.float32, kind="ExternalInput")
    be_d = nc.dram_tensor("beta_b", [128, 128], dt.float32, kind="ExternalInput")
    id_d = nc.dram_tensor("ident", [128, 128], dt.bfloat16, kind="ExternalInput")
    y_d = nc.dram_tensor("y", [DPAD, 128], dt.float32, kind="ExternalOutput")

    with tile.TileContext(nc) as tc, ExitStack() as ctx:
        consts = ctx.enter_context(tc.tile_pool(name="consts", bufs=1))
        xpool = ctx.enter_context(tc.tile_pool(name="xin", bufs=2))
        qpool = ctx.enter_context(tc.tile_pool(name="qp", bufs=2))
        ppool = ctx.enter_context(tc.tile_pool(name="pp", bufs=2))
        spool = ctx.enter_context(tc.tile_pool(name="small", bufs=3))
        kpool = ctx.enter_context(tc.tile_pool(name="kblk", bufs=2))
        opool = ctx.enter_context(tc.tile_pool(name="outs", bufs=3))
        # PSUM budget (8 banks): psq 3 + psv 2 + accT 1 + kps/oproj 2
        psq_p = ctx.enter_context(tc.tile_pool(name="psq", bufs=3, space="PSUM"))
        psv_p = ctx.enter_context(tc.tile_pool(name="psv", bufs=2, space="PSUM"))
        psacc = ctx.enter_context(tc.tile_pool(name="psacc", bufs=1, space="PSUM"))
        psmisc = ctx.enter_context(tc.tile_pool(name="psmisc", bufs=1, space="PSUM"))

        # ---------- constants ----------
        wq = consts.tile([128, 128], dt.bfloat16)
        wk = consts.tile([128, 128], dt.bfloat16)
        wvt = consts.tile([128, 128], dt.bfloat16)
        wo = consts.tile([128, 128], dt.bfloat16)
        bo = consts.tile([128, 128], dt.float32)
        ga = consts.tile([128, 128], dt.float32)
        be = consts.tile([128, 128], dt.float32)
        ident = consts.tile([128, 128], dt.bfloat16)
        epsT = consts.tile([128, 1], dt.float32)
        for dst, src in ((wq, wq_d), (wk, wk_d), (wvt, wv_d), (wo, wo_d),
                         (bo, bo_d), (ga, ga_d), (be, be_d), (ident, id_d)):
            nc.sync.dma_start(out=dst[:], in_=src[:])
        nc.vector.memset(epsT[:], LN_EPS)

        # deferred-LN collection buffers (persist across the block loop)
        y2a = consts.tile([128, NB, 128], dt.float32)
        mva = consts.tile([128, NB, 2], dt.float32)

        # per-block state carried stage1 -> stage2
        state = [None] * NB

        def stage1(b):
            nr = R[b]
            g0 = r_off[b]
            st = {}
            # K for this block (1/sqrt(HD) pre-folded into wk on host)
            xtd = kpool.tile([128, 128], dt.bfloat16, tag="xtd")
            nc.sync.dma_start(out=xtd[:], in_=xtd_d[:, b * 128:(b + 1) * 128])
            kps = psmisc.tile([128, 128], dt.float32, tag="kps")
            nc.tensor.matmul(kps[:], xtd[:], wk[:], start=True, stop=True)
            kd = kpool.tile([128, 128], dt.bfloat16, tag="kd")
            nc.scalar.copy(out=kd[:], in_=kps[:])
            st["kd"] = kd
            xdt_ = opool.tile([128, 128], dt.float32, tag="xdt")
            nc.sync.dma_start(out=xdt_[:], in_=xd_d[b * 128:(b + 1) * 128, :])
            st["xd"] = xdt_

            # inputs for the whole block
            xt = xpool.tile([128, RMAX * 128], dt.bfloat16, tag="xt")
            nc.sync.dma_start(out=xt[:, :nr * 128],
                              in_=xdt_d[:, g0 * 128:(g0 + nr) * 128])
            eat = xpool.tile([128, RMAX, 16], dt.bfloat16, tag="eat")
            nc.sync.dma_start(out=eat[:, :nr, :],
                              in_=ea_d[:, g0 * 16:(g0 + nr) * 16]
                              .rearrange("p (r s) -> p r s", s=16))

            # Q projection per round; drain PSUM via ACT.  (V runs in stage2
            # so its PSUM drain fuses into the probs multiply on DVE.)
            qs = qpool.tile([128, RMAX, 128], dt.bfloat16, tag="qs")
            r0 = 0
            while r0 < nr:
                gn = min(GROUP, nr - r0)
                psq = psq_p.tile([128, GROUP, 128], dt.float32, tag="psq")
                for r in range(gn):
                    lhs = xt[:, (r0 + r) * 128:(r0 + r + 1) * 128]
                    nc.tensor.matmul(psq[:, r, :], lhs, wq[:], start=True, stop=True)
                nc.scalar.copy(out=qs[:, r0:r0 + gn, :], in_=psq[:, :gn, :])
                r0 += gn
            st["xt"], st["eat"], st["qs"] = xt, eat, qs
            state[b] = st

        def stage2(b):
            nr = R[b]
            st = state[b]
            xt, eat, qs, kd = st["xt"], st["eat"], st["qs"], st["kd"]
            c = nr

            # edge weight: ew = sigmoid(sum ea) via Exp-only path
            easum = spool.tile([128, RMAX], dt.float32, tag="easum")
            nc.vector.tensor_reduce(easum[:, :c], eat[:, :c, :],
                                    axis=mybir.AxisListType.X, op=AL.add,
                                    negate=True)
            een = spool.tile([128, RMAX], dt.float32, tag="een")
            nc.scalar.activation(out=een[:, :c], in_=easum[:, :c], func=AF.Exp)
            ew1 = spool.tile([128, RMAX], dt.float32, tag="ew1")
            nc.vector.tensor_scalar_add(ew1[:, :c], een[:, :c], 1.0)
            ew = spool.tile([128, RMAX], dt.float32, tag="ew")
            nc.vector.reciprocal(out=ew[:, :c], in_=ew1[:, :c])

            # prod = Q * K (K broadcast over rounds) on the Pool engine; the
            # first tree level (t1) also runs there.  DVE keeps the rest.
            prod = ppool.tile([128, RMAX, 128], dt.bfloat16, tag="prod")
            kb0 = bass.AP(tensor=kd.tensor, offset=kd.offset,
                          ap=[list(kd.ap[0]), [0, c], [1, 128]])
            nc.gpsimd.tensor_tensor(out=prod[:, :c, :], in0=qs[:, :c, :],
                                    in1=kb0, op=AL.mult)
            # head reduce via pairwise tree
            p4 = prod[:, :c, :].rearrange("p c (h s) -> p c h s", s=16)
            t1_ = ppool.tile([128, RMAX, 8, 8], dt.bfloat16, tag="tr1")
            nc.gpsimd.tensor_tensor(out=t1_[:, :c, :, :], in0=p4[:, :, :, 0:8],
                                    in1=p4[:, :, :, 8:16], op=AL.add)
            t2_ = spool.tile([128, RMAX, 8, 4], dt.bfloat16, tag="tr2")
            nc.vector.tensor_tensor(out=t2_[:, :c, :, :], in0=t1_[:, :c, :, 0:4],
                                    in1=t1_[:, :c, :, 4:8], op=AL.add)
            t3_ = spool.tile([128, RMAX, 8, 2], dt.bfloat16, tag="tr3")
            nc.vector.tensor_tensor(out=t3_[:, :c, :, :], in0=t2_[:, :c, :, 0:2],
                                    in1=t2_[:, :c, :, 2:4], op=AL.add)
            sraw = spool.tile([128, RMAX, 8], dt.float32, tag="sraw")
            nc.vector.tensor_tensor(out=sraw[:, :c, :], in0=t3_[:, :c, :, 0],
                                    in1=t3_[:, :c, :, 1], op=AL.add)
            # leaky relu: max(alpha*x, x)
            slr = spool.tile([128, RMAX, 8], dt.float32, tag="slr")
            nc.vector.scalar_tensor_tensor(out=slr[:, :c, :], in0=sraw[:, :c, :],
                                           scalar=ALPHA, in1=sraw[:, :c, :],
                                           op0=AL.mult, op1=AL.max)
            # * edge weight (bcast over heads)
            ewb = bass.AP(tensor=ew.tensor, offset=ew.offset,
                          ap=[list(ew.ap[0]), [1, c], [0, 8]])
            sw = spool.tile([128, RMAX, 8], dt.float32, tag="sw")
            nc.vector.tensor_tensor(out=sw[:, :c, :], in0=slr[:, :c, :],
                                    in1=ewb, op=AL.mult)
            # exp (scores are small; no max-sub needed)
            esc = spool.tile([128, RMAX, 8], dt.float32, tag="esc")
            nc.scalar.activation(out=esc[:, :c, :], in_=sw[:, :c, :], func=AF.Exp)
            # sum over heads + reciprocal
            ses = spool.tile([128, RMAX], dt.float32, tag="ses")
            nc.vector.tensor_reduce(ses[:, :c], esc[:, :c, :],
                                    axis=mybir.AxisListType.X, op=AL.add)
            rec = spool.tile([128, RMAX], dt.float32, tag="rec")
            nc.vector.reciprocal(out=rec[:, :c], in_=ses[:, :c])
            # probs = esc * rec (bcast over heads) -> bf16
            rcb = bass.AP(tensor=rec.tensor, offset=rec.offset,
                          ap=[list(rec.ap[0]), [1, c], [0, 8]])
            probs = spool.tile([128, RMAX, 8], dt.bfloat16, tag="probs")
            nc.vector.tensor_tensor(out=probs[:, :c, :], in0=esc[:, :c, :],
                                    in1=rcb, op=AL.mult)

            # V projection per group; PSUM drain fused into the probs
            # multiply: wv = V_psum * probs (bcast 16 within head) -> bf16
            accT = psacc.tile([128, 128], dt.float32, tag="accT", name="accT")
            wvt_t = ppool.tile([128, RMAX, 128], dt.bfloat16, tag="wv")
            r0 = 0
            while r0 < c:
                gn = min(GROUP, c - r0)
                psv = psv_p.tile([128, GROUP, 128], dt.float32, tag="psv")
                for r in range(gn):
                    lhs = xt[:, (r0 + r) * 128:(r0 + r + 1) * 128]
                    nc.tensor.matmul(psv[:, r, :], lhs, wvt[:], start=True, stop=True)
                pb = bass.AP(tensor=probs.tensor, offset=probs.offset + r0 * 8,
                             ap=[list(probs.ap[0]), [8, gn], [1, 8], [0, 16]])
                nc.vector.tensor_tensor(out=wvt_t[:, r0:r0 + gn, :],
                                        in0=psv[:, :gn, :], in1=pb, op=AL.mult)
                # accumulate transposed: accT += wv_r.T  (stationary wv_r,
                # streaming identity) so the output stage needs no transpose
                for r in range(gn):
                    i = r0 + r
                    nc.tensor.matmul(accT[:], wvt_t[:, i, :], ident[:],
                                     start=(i == 0), stop=(i == c - 1),
                                     skip_group_check=True)
                r0 += gn

            # ---------- output stage (LN sqrt deferred + batched) ----------
            accTs = opool.tile([128, 128], dt.bfloat16, tag="accTs")
            nc.scalar.copy(out=accTs[:], in_=accT[:])
            oproj = psmisc.tile([128, 128], dt.float32, tag="oproj")
            nc.tensor.matmul(oproj[:], accTs[:], wo[:], start=True, stop=True)

            nc.vector.tensor_tensor(out=y2a[:, b, :], in0=oproj[:],
                                    in1=st["xd"][:], op=AL.add)
            stt = spool.tile([128, 6], dt.float32, tag="st")
            nc.vector.bn_stats(out=stt[:], in_=y2a[:, b, :])
            nc.vector.bn_aggr(out=mva[:, b, :], in_=stt[:])

        # software-pipelined emission: stage1 one block ahead of stage2
        stage1(0)
        for b in range(NB):
            if b + 1 < NB:
                stage1(b + 1)
            stage2(b)

        # batched LN: one sqrt + reciprocal for all blocks
        sd = consts.tile([128, NB], dt.float32)
        nc.scalar.activation(out=sd[:], in_=mva[:, :, 1], func=AF.Sqrt,
                             bias=epsT[:])
        rstd = consts.tile([128, NB], dt.float32)
        nc.vector.reciprocal(out=rstd[:], in_=sd[:])
        for b in range(NB):
            t1 = opool.tile([128, 128], dt.float32, tag="t1")
            nc.vector.scalar_tensor_tensor(out=t1[:], in0=y2a[:, b, :],
                                           scalar=mva[:, b, 0:1], in1=ga[:],
                                           op0=AL.subtract, op1=AL.mult)
            yn = opool.tile([128, 128], dt.float32, tag="yn")
            nc.vector.scalar_tensor_tensor(out=yn[:], in0=t1[:],
                                           scalar=rstd[:, b:b + 1], in1=be[:],
                                           op0=AL.mult, op1=AL.add)
            nc.sync.dma_start(out=y_d[b * 128:(b + 1) * 128, :], in_=yn[:])

    nc.compile()
    return nc


# ------------------------------------------------------------------ runner
def _in_maps(meta, arrs, w):
    NC = meta["cfg"]["NC"]
    bo_row = w["bo_b"][0][None, :]     # bias folded into the residual input
    maps = []
    for c in range(NC):
        maps.append(dict(
            xdt=np.ascontiguousarray(arrs["xdt"][c]),
            ea=np.ascontiguousarray(arrs["ea"][c]),
            xtd=np.ascontiguousarray(arrs["xtd"][c]),
            xd=np.ascontiguousarray(arrs["xd"][c] + bo_row),
            **{k: np.ascontiguousarray(v) for k, v in w.items()},
        ))
    return maps


def assemble(meta, arrs, results):
    cfg = meta["cfg"]
    N, NC, DPC = cfg["N"], cfg["NC"], meta["DPC"]
    out = np.empty((N, D), dtype=np.float32)
    for c in range(NC):
        yc = results[c]["y"]
        perm = arrs["perms"][c]
        valid = perm >= 0
        out[c * DPC + perm[valid]] = yc[:meta["DPAD"]][valid]
    return out


_CACHE = {}


def kernel(x, edge_index, edge_attr, Wq, Wk, Wv, Wo, bo, gamma, beta):
    cfg = FULL_CFG
    meta, arrs = host_prep(x, edge_index, edge_attr, cfg)
    w = host_weights(Wq, Wk, Wv, Wo, bo, gamma, beta)
    key = tuple(meta["R"])
    if key not in _CACHE:
        _CACHE[key] = build_nc(meta)
    nc = _CACHE[key]
    from concourse.bass_utils import run_bass_kernel_spmd
    res = run_bass_kernel_spmd(nc, _in_maps(meta, arrs, w),
                               core_ids=list(range(cfg["NC"])))
    return assemble(meta, arrs, res.results)


if __name__ == "__main__":
    import reference
    inputs = {k: np.asarray(v) for k, v in reference.setup_inputs().items()}
    out = kernel(**inputs)
    exp = np.asarray(reference.reference(**reference.setup_inputs()))
    err = np.abs(out - exp).max() / max(np.abs(exp).max(), 1e-9)
    print("Relative error:", err)


# revision 22
# speedup vs baseline: 1.2385x; 1.2385x over previous
"""CrossDomainGAT Trainium2 kernel — gatherless edge-slot design.

Strategy (graph/data parallel per the sharding hint):
  - Destination nodes sharded across 8 cores (6250 dests/core, padded to
    6272 = 49 blocks x 128).  Edges are routed to the core owning the
    destination, so the per-edge softmax (over heads -- edge-local) and the
    scatter-add stay local.
  - NO on-device gather.  The host pre-gathers x^T into *edge-slot* order
    (dest-major rounds: slot s = r*128 + p holds the r-th in-edge of dest p
    of its block; dests are degree-sorted so blocks have uniform round
    counts).  The device computes Q and V *per edge slot* on the Tensor
    engine: per round, the 128-column x^T tile is the stationary operand and
    Wq^T / Wv^T stream through.  This replaces the baseline's
    dma_gather-based pipeline whose SWDGE descriptor generation saturated
    GpSimd (~8 ns/edge) and whose 512B-packet storm stalled DVE.
  - K for a block is computed once from the dest rows and broadcast across
    rounds via a stride-0 access pattern (dests are partition-aligned).
  - Per-edge math runs dest-major at BLOCK granularity (few, large ops):
    ACT drains Q PSUM; the Q*K product is split between DVE and the
    otherwise-idle Pool engine; the head-sum is a single 16->1 tensor_reduce;
    V's PSUM drain is fused into the probs multiply on DVE.
  - The scatter-add accumulates acc^T directly on the TensorEngine
    (stationary wv_r, streaming identity), so the output stage needs no
    transpose round-trip.  LN sqrt is deferred and batched; host un-permutes.
  - Emission is software-pipelined one block ahead (stage1: DMA + K + Q
    matmuls + drains; stage2: edge math + V + accumulation + output).
"""

import math
import numpy as np
import ml_dtypes

# ---------------------------------------------------------------- problem cfg
D = 128
H = 8
HD = 16
ALPHA = 0.2
LN_EPS = 1e-5

FULL_CFG = dict(
    N=50000,
    E=800000,
    NC=8,            # cores
    GROUP=4,         # rounds per PSUM bank group
    POOL_FRAC=0.45,  # fraction of prod rounds computed on the Pool engine
)


def host_prep(x, edge_index, edge_attr, cfg):
    """Route edges to dest cores, degree-sort dests into uniform blocks,
    and materialize x^T in edge-slot order (plus per-block dest tensors)."""
    N, E, NC = cfg["N"], cfg["E"], cfg["NC"]
    DPC = N // NC                      # dests per core
    NB = (DPC + 127) // 128            # dest blocks per core
    DPAD = NB * 128

    row = np.asarray(edge_index[0], dtype=np.int64)
    col = np.asarray(edge_index[1], dtype=np.int64)
    ea = np.asarray(edge_attr, dtype=np.float32)
    x = np.asarray(x, dtype=np.float32)
    bf = ml_dtypes.bfloat16

    core = col // DPC
    cl = col - core * DPC              # local dest id

    # ---- pass 1: per-core degree sort -> uniform per-block round counts
    perms, orders = [], []
    Rc = np.zeros((NC, NB), dtype=np.int64)
    for c in range(NC):
        clc = cl[core == c]
        deg = np.bincount(clc, minlength=DPC)
        order = np.argsort(-deg, kind="stable")
        dpad = np.concatenate([deg[order], np.zeros(DPAD - DPC, np.int64)])
        Rc[c] = dpad.reshape(NB, 128).max(axis=1)
        perm = np.concatenate([order, np.full(DPAD - DPC, -1, np.int64)])
        perms.append(perm)
        orders.append(order)

    R = np.maximum(Rc.max(axis=0), 1)  # uniform across cores, >=1
    r_off = np.concatenate([[0], np.cumsum(R)])
    NR = int(R.sum())                  # total rounds per core
    S = NR * 128                       # edge slots per core

    # x^T padded with one zero column for pad slots
    xT = np.ascontiguousarray(x.T).astype(bf)
    xTpad = np.concatenate([xT, np.zeros((D, 1), dtype=bf)], axis=1)

    xdt_arrs, ea_arrs, xtd_arrs, xd_arrs = [], [], [], []
    for c in range(NC):
        m = core == c
        clc = cl[m]
        rowc = row[m]
        eac = ea[m]
        order = orders[c]
        inv = np.empty(DPC, dtype=np.int64)
        inv[order] = np.arange(DPC)
        q = inv[clc]                   # dest slot of each edge
        b = q // 128
        p = q % 128
        # rank within dest via stable sort on dest slot
        sort = np.argsort(q, kind="stable")
        qs = q[sort]
        starts = np.r_[0, np.flatnonzero(np.diff(qs)) + 1]
        counts = np.diff(np.r_[starts, len(qs)])
        rank_sorted = np.arange(len(qs)) - np.repeat(starts, counts)
        rank = np.empty(len(qs), np.int64)
        rank[sort] = rank_sorted

        s_idx = (r_off[b] + rank) * 128 + p
        src_col = np.full(S, N, dtype=np.int64)  # default: the zero column
        src_col[s_idx] = rowc
        xdt_arrs.append(np.ascontiguousarray(xTpad[:, src_col]))

        ea_l = np.zeros((S, 16), dtype=np.float32)
        ea_l[s_idx] = eac
        ea_arrs.append(np.ascontiguousarray(
            ea_l.reshape(NR, 128, 16).transpose(1, 0, 2).reshape(128, NR * 16)
        ).astype(bf))

        perm = perms[c]
        xd = np.zeros((DPAD, D), dtype=np.float32)
        valid = perm >= 0
        xd[valid] = x[c * DPC + perm[valid]]
        xtd_arrs.append(np.ascontiguousarray(xd.T).astype(bf))
        xd_arrs.append(xd)  # bias bo is folded in by kernel() before upload

    meta = dict(
        cfg=cfg, DPC=DPC, NB=NB, DPAD=DPAD,
        R=R.astype(int).tolist(), NR=NR, S=S,
        r_off=r_off.astype(int).tolist(),
        n_rounds=NR,
    )
    arrs = dict(
        xdt=xdt_arrs, ea=ea_arrs, xtd=xtd_arrs, xd=xd_arrs, perms=perms,
    )
    return meta, arrs


# ------------------------------------------------------------------ weights
def host_weights(Wq, Wk, Wv, Wo, bo, gamma, beta):
    bf = ml_dtypes.bfloat16
    t = lambda W: np.ascontiguousarray(np.asarray(W, np.float32).T).astype(bf)
    rep = lambda v: np.tile(np.asarray(v, np.float32)[None, :], (128, 1))
    # 1/sqrt(HD) folded into Wk so K needs no on-device scale
    wk_s = np.asarray(Wk, np.float32) / math.sqrt(HD)
    return dict(
        wq_t=t(Wq), wk_t=t(wk_s), wv_t=t(Wv), wo_t=t(Wo),
        bo_b=rep(bo), gamma_b=rep(gamma), beta_b=rep(beta),
        ident=np.eye(128, dtype=np.float32).astype(bf),
    )


# ------------------------------------------------------------------ kernel IR
def build_nc(meta, debug=False):
    from contextlib import ExitStack
    import concourse.bacc as bacc
    import concourse.bass as bass
    import concourse.tile as tile
    from concourse import mybir

    cfg = meta["cfg"]
    NB, DPAD = meta["NB"], meta["DPAD"]
    R = meta["R"]
    NR, S = meta["NR"], meta["S"]
    r_off = meta["r_off"]
    GROUP = cfg["GROUP"]
    POOL_FRAC = cfg["POOL_FRAC"]
    RMAX = max(R)

    dt = mybir.dt
    AF = mybir.ActivationFunctionType
    AL = mybir.AluOpType

    nc = bacc.Bacc("TRN2", target_bir_lowering=False, debug=debug)

    # ---------- I/O ----------
    xdt_d = nc.dram_tensor("xdt", [128, S], dt.bfloat16, kind="ExternalInput")
    ea_d = nc.dram_tensor("ea", [128, NR * 16], dt.bfloat16, kind="ExternalInput")
    xtd_d = nc.dram_tensor("xtd", [128, DPAD], dt.bfloat16, kind="ExternalInput")
    xd_d = nc.dram_tensor("xd", [DPAD, 128], dt.float32, kind="ExternalInput")
    wq_d = nc.dram_tensor("wq_t", [128, 128], dt.bfloat16, kind="ExternalInput")
    wk_d = nc.dram_tensor("wk_t", [128, 128], dt.bfloat16, kind="ExternalInput")
    wv_d = nc.dram_tensor("wv_t", [128, 128], dt.bfloat16, kind="ExternalInput")
    wo_d = nc.dram_tensor("wo_t", [128, 128], dt.bfloat16, kind="ExternalInput")
    bo_d = nc.dram_tensor("bo_b", [128, 128], dt.float32, kind="ExternalInput")
    ga_d = nc.dram_tensor("gamma_b", [128, 128], dt.float32, kind="ExternalInput")
    be_d = nc.dram_tensor("beta_b", [128, 128], dt.float32, kind="ExternalInput")
    id_d = nc.dram_tensor("ident", [128, 128], dt.bfloat16, kind="ExternalInput")
    y_d = nc.dram_tensor("y", [DPAD, 128], dt.float32, kind="ExternalOutput")

    with tile.TileContext(nc) as tc, ExitStack() as ctx:
        consts = ctx.enter_context(tc.tile_pool(name="consts", bufs=1))
        xpool = ctx.enter_context(tc.tile_pool(name="xin", bufs=2))
        ppool = ctx.enter_context(tc.tile_pool(name="pp", bufs=2))
        spool = ctx.enter_context(tc.tile_pool(name="small", bufs=3))
        kpool = ctx.enter_context(tc.tile_pool(name="kblk", bufs=2))
        opool = ctx.enter_context(tc.tile_pool(name="outs", bufs=3))
        # PSUM budget (8 banks): psq 3 + psv 2 + accT 1 + kps/oproj 2
        psq_p = ctx.enter_context(tc.tile_pool(name="psq", bufs=3, space="PSUM"))
        psv_p = ctx.enter_context(tc.tile_pool(name="psv", bufs=2, space="PSUM"))
        psacc = ctx.enter_context(tc.tile_pool(name="psacc", bufs=1, space="PSUM"))
        psmisc = ctx.enter_context(tc.tile_pool(name="psmisc", bufs=1, space="PSUM"))

        # ---------- constants ----------
        wq = consts.tile([128, 128], dt.bfloat16)
        wk = consts.tile([128, 128], dt.bfloat16)
        wvt = consts.tile([128, 128], dt.bfloat16)
        wo = consts.tile([128, 128], dt.bfloat16)
        bo = consts.tile([128, 128], dt.float32)
        ga = consts.tile([128, 128], dt.float32)
        be = consts.tile([128, 128], dt.float32)
        ident = consts.tile([128, 128], dt.bfloat16)
        epsT = consts.tile([128, 1], dt.float32)
        for dst, src in ((wq, wq_d), (wk, wk_d), (wvt, wv_d), (wo, wo_d),
                         (bo, bo_d), (ga, ga_d), (be, be_d), (ident, id_d)):
            nc.sync.dma_start(out=dst[:], in_=src[:])
        nc.vector.memset(epsT[:], LN_EPS)

        # per-block state carried stage1 -> stage2
        state = [None] * NB

        def stage1(b):
            nr = R[b]
            g0 = r_off[b]
            st = {}
            # K for this block (1/sqrt(HD) pre-folded into wk on host)
            xtd = kpool.tile([128, 128], dt.bfloat16, tag="xtd")
            nc.sync.dma_start(out=xtd[:], in_=xtd_d[:, b * 128:(b + 1) * 128])
            kps = psmisc.tile([128, 128], dt.float32, tag="kps")
            nc.tensor.matmul(kps[:], xtd[:], wk[:], start=True, stop=True)
            kd = kpool.tile([128, 128], dt.bfloat16, tag="kd")
            nc.scalar.copy(out=kd[:], in_=kps[:])
            st["kd"] = kd
            xdt_ = opool.tile([128, 128], dt.float32, tag="xdt")
            nc.sync.dma_start(out=xdt_[:], in_=xd_d[b * 128:(b + 1) * 128, :])
            st["xd"] = xdt_

            # inputs for the whole block
            xt = xpool.tile([128, RMAX * 128], dt.bfloat16, tag="xt")
            nc.sync.dma_start(out=xt[:, :nr * 128],
                              in_=xdt_d[:, g0 * 128:(g0 + nr) * 128])
            eat = xpool.tile([128, RMAX, 16], dt.bfloat16, tag="eat")
            nc.sync.dma_start(out=eat[:, :nr, :],
                              in_=ea_d[:, g0 * 16:(g0 + nr) * 16]
                              .rearrange("p (r s) -> p r s", s=16))

            # Q projection per round; the PSUM drain is fused into the Q*K
            # product on DVE (prod = psq * K_bcast -> bf16), so no ACT copy.
            prod = ppool.tile([128, RMAX, 128], dt.bfloat16, tag="prod")
            r0 = 0
            while r0 < nr:
                gn = min(GROUP, nr - r0)
                psq = psq_p.tile([128, GROUP, 128], dt.float32, tag="psq")
                for r in range(gn):
                    lhs = xt[:, (r0 + r) * 128:(r0 + r + 1) * 128]
                    nc.tensor.matmul(psq[:, r, :], lhs, wq[:], start=True, stop=True)
                kb = bass.AP(tensor=kd.tensor, offset=kd.offset,
                             ap=[list(kd.ap[0]), [0, gn], [1, 128]])
                nc.vector.tensor_tensor(out=prod[:, r0:r0 + gn, :],
                                        in0=psq[:, :gn, :], in1=kb, op=AL.mult)
                r0 += gn
            st["xt"], st["eat"], st["prod"] = xt, eat, prod
            state[b] = st

        def stage2(b):
            nr = R[b]
            st = state[b]
            xt, eat, prod = st["xt"], st["eat"], st["prod"]
            c = nr

            # edge weight: ew = sigmoid(sum ea) via Exp-only path
            easum = spool.tile([128, RMAX], dt.float32, tag="easum")
            nc.vector.tensor_reduce(easum[:, :c], eat[:, :c, :],
                                    axis=mybir.AxisListType.X, op=AL.add,
                                    negate=True)
            een = spool.tile([128, RMAX], dt.float32, tag="een")
            nc.scalar.activation(out=een[:, :c], in_=easum[:, :c], func=AF.Exp)
            ew1 = spool.tile([128, RMAX], dt.float32, tag="ew1")
            nc.vector.tensor_scalar_add(ew1[:, :c], een[:, :c], 1.0)
            ew = spool.tile([128, RMAX], dt.float32, tag="ew")
            nc.vector.reciprocal(out=ew[:, :c], in_=ew1[:, :c])

            # head reduce via pairwise tree
            p4 = prod[:, :c, :].rearrange("p c (h s) -> p c h s", s=16)
            t1_ = ppool.tile([128, RMAX, 8, 8], dt.bfloat16, tag="tr1")
            nc.vector.tensor_tensor(out=t1_[:, :c, :, :], in0=p4[:, :, :, 0:8],
                                    in1=p4[:, :, :, 8:16], op=AL.add)
            t2_ = spool.tile([128, RMAX, 8, 4], dt.bfloat16, tag="tr2")
            nc.vector.tensor_tensor(out=t2_[:, :c, :, :], in0=t1_[:, :c, :, 0:4],
                                    in1=t1_[:, :c, :, 4:8], op=AL.add)
            t3_ = spool.tile([128, RMAX, 8, 2], dt.bfloat16, tag="tr3")
            nc.vector.tensor_tensor(out=t3_[:, :c, :, :], in0=t2_[:, :c, :, 0:2],
                                    in1=t2_[:, :c, :, 2:4], op=AL.add)
            sraw = spool.tile([128, RMAX, 8], dt.float32, tag="sraw")
            nc.vector.tensor_tensor(out=sraw[:, :c, :], in0=t3_[:, :c, :, 0],
                                    in1=t3_[:, :c, :, 1], op=AL.add)
            # leaky relu: max(alpha*x, x)
            slr = spool.tile([128, RMAX, 8], dt.float32, tag="slr")
            nc.vector.scalar_tensor_tensor(out=slr[:, :c, :], in0=sraw[:, :c, :],
                                           scalar=ALPHA, in1=sraw[:, :c, :],
                                           op0=AL.mult, op1=AL.max)
            # * edge weight (bcast over heads)
            ewb = bass.AP(tensor=ew.tensor, offset=ew.offset,
                          ap=[list(ew.ap[0]), [1, c], [0, 8]])
            sw = spool.tile([128, RMAX, 8], dt.float32, tag="sw")
            nc.vector.tensor_tensor(out=sw[:, :c, :], in0=slr[:, :c, :],
                                    in1=ewb, op=AL.mult)
            # exp (scores are small; no max-sub needed)
            esc = spool.tile([128, RMAX, 8], dt.float32, tag="esc")
            nc.scalar.activation(out=esc[:, :c, :], in_=sw[:, :c, :], func=AF.Exp)
            # sum over heads + reciprocal
            ses = spool.tile([128, RMAX], dt.float32, tag="ses")
            nc.vector.tensor_reduce(ses[:, :c], esc[:, :c, :],
                                    axis=mybir.AxisListType.X, op=AL.add)
            rec = spool.tile([128, RMAX], dt.float32, tag="rec")
            nc.vector.reciprocal(out=rec[:, :c], in_=ses[:, :c])
            # probs = esc * rec (bcast over heads) -> bf16
            rcb = bass.AP(tensor=rec.tensor, offset=rec.offset,
                          ap=[list(rec.ap[0]), [1, c], [0, 8]])
            probs = spool.tile([128, RMAX, 8], dt.bfloat16, tag="probs")
            nc.vector.tensor_tensor(out=probs[:, :c, :], in0=esc[:, :c, :],
                                    in1=rcb, op=AL.mult)

            # V projection per group; PSUM drain fused into the probs
            # multiply: wv = V_psum * probs (bcast 16 within head) -> bf16
            accT = psacc.tile([128, 128], dt.float32, tag="accT", name="accT")
            wvt_t = ppool.tile([128, RMAX, 128], dt.bfloat16, tag="wv")
            r0 = 0
            while r0 < c:
                gn = min(GROUP, c - r0)
                psv = psv_p.tile([128, GROUP, 128], dt.float32, tag="psv")
                for r in range(gn):
                    lhs = xt[:, (r0 + r) * 128:(r0 + r + 1) * 128]
                    nc.tensor.matmul(psv[:, r, :], lhs, wvt[:], start=True, stop=True)
                pb = bass.AP(tensor=probs.tensor, offset=probs.offset + r0 * 8,
                             ap=[list(probs.ap[0]), [8, gn], [1, 8], [0, 16]])
                nc.vector.tensor_tensor(out=wvt_t[:, r0:r0 + gn, :],
                                        in0=psv[:, :gn, :], in1=pb, op=AL.mult)
                # accumulate transposed: accT += wv_r.T  (stationary wv_r,
                # streaming identity) so the output stage needs no transpose
                for r in range(gn):
                    i = r0 + r
                    nc.tensor.matmul(accT[:], wvt_t[:, i, :], ident[:],
                                     start=(i == 0), stop=(i == c - 1),
                                     skip_group_check=True)
                r0 += gn

            # ---------- output stage (LN sqrt deferred + batched) ----------
            accTs = opool.tile([128, 128], dt.bfloat16, tag="accTs")
            nc.scalar.copy(out=accTs[:], in_=accT[:])
            oproj = psmisc.tile([128, 128], dt.float32, tag="oproj")
            nc.tensor.matmul(oproj[:], accTs[:], wo[:], start=True, stop=True)

            y2 = opool.tile([128, 128], dt.float32, tag="y2")
            nc.vector.tensor_tensor(out=y2[:], in0=oproj[:],
                                    in1=st["xd"][:], op=AL.add)
            stt = spool.tile([128, 6], dt.float32, tag="st")
            nc.vector.bn_stats(out=stt[:], in_=y2[:])
            mv = spool.tile([128, 2], dt.float32, tag="mv")
            nc.vector.bn_aggr(out=mv[:], in_=stt[:])
            sd = spool.tile([128, 1], dt.float32, tag="sd")
            nc.scalar.activation(out=sd[:], in_=mv[:, 1:2], func=AF.Sqrt,
                                 bias=epsT[:])
            rstd = spool.tile([128, 1], dt.float32, tag="rstd")
            nc.vector.reciprocal(out=rstd[:], in_=sd[:])
            t1 = opool.tile([128, 128], dt.float32, tag="t1")
            nc.vector.scalar_tensor_tensor(out=t1[:], in0=y2[:],
                                           scalar=mv[:, 0:1], in1=ga[:],
                                           op0=AL.subtract, op1=AL.mult)
            yn = opool.tile([128, 128], dt.float32, tag="yn")
            nc.vector.scalar_tensor_tensor(out=yn[:], in0=t1[:],
                                           scalar=rstd[:], in1=be[:],
                                           op0=AL.mult, op1=AL.add)
            nc.sync.dma_start(out=y_d[b * 128:(b + 1) * 128, :], in_=yn[:])

        # software-pipelined emission: stage1 one block ahead of stage2
        stage1(0)
        for b in range(NB):
            if b + 1 < NB:
                stage1(b + 1)
            stage2(b)

    nc.compile()
    return nc


# ------------------------------------------------------------------ runner
def _in_maps(meta, arrs, w):
    NC = meta["cfg"]["NC"]
    bo_row = w["bo_b"][0][None, :]     # bias folded into the residual input
    maps = []
    for c in range(NC):
        maps.append(dict(
            xdt=np.ascontiguousarray(arrs["xdt"][c]),
            ea=np.ascontiguousarray(arrs["ea"][c]),
            xtd=np.ascontiguousarray(arrs["xtd"][c]),
            xd=np.ascontiguousarray(arrs["xd"][c] + bo_row),
            **{k: np.ascontiguousarray(v) for k, v in w.items()},
        ))
    return maps


def assemble(meta, arrs, results):
    cfg = meta["cfg"]
    N, NC, DPC = cfg["N"], cfg["NC"], meta["DPC"]
    out = np.empty((N, D), dtype=np.float32)
    for c in range(NC):
        yc = results[c]["y"]
        perm = arrs["perms"][c]
        valid = perm >= 0
        out[c * DPC + perm[valid]] = yc[:meta["DPAD"]][valid]
    return out


_CACHE = {}


def kernel(x, edge_index, edge_attr, Wq, Wk, Wv, Wo, bo, gamma, beta):
    cfg = FULL_CFG
    meta, arrs = host_prep(x, edge_index, edge_attr, cfg)
    w = host_weights(Wq, Wk, Wv, Wo, bo, gamma, beta)
    key = tuple(meta["R"])
    if key not in _CACHE:
        _CACHE[key] = build_nc(meta)
    nc = _CACHE[key]
    from concourse.bass_utils import run_bass_kernel_spmd
    res = run_bass_kernel_spmd(nc, _in_maps(meta, arrs, w),
                               core_ids=list(range(cfg["NC"])))
    return assemble(meta, arrs, res.results)


if __name__ == "__main__":
    import reference
    inputs = {k: np.asarray(v) for k, v in reference.setup_inputs().items()}
    out = kernel(**inputs)
    exp = np.asarray(reference.reference(**reference.setup_inputs()))
    err = np.abs(out - exp).max() / max(np.abs(exp).max(), 1e-9)
    print("Relative error:", err)
